# revision 6
# baseline (speedup 1.0000x reference)
"""FP4 block-quantized linear: y = x @ dequant(packed, scales, zeros).T + bias.

Tensor-parallel over out_features across 8 NeuronCores (1536 rows each).

Per-core device pipeline, per 128x128 weight tile:
  - DVE: extract hi/lo nibbles of the packed bytes into the two contiguous
    halves of a [128o, 128i'] tile (i' = even-i then odd-i permutation),
    fused with a per-partition add of zos = zeros/scales.
  - ACT: build diag(scales) from a constant identity via per-partition scale.
  - PE:  psum[i', o'] = qn[o, i'].T @ diag(s)[o, o']  -- one regular N=128
    matmul performs transpose AND scale: W.T tile = (q + z/s) * s = q*s + z.
  - evict psum -> SBUF fp16 stationary tile.
Main matmul accumulates y.T[o, t] over the 32 K-blocks in PSUM (N=512),
software-pipelined so dequant matmuls of the next row-block interleave 1:4
with main matmuls. x arrives as x.T in fp16 with rows permuted to match i'.
The host undoes nothing on the output path except transpose/concat + bias.
"""

import numpy as np

OUT, IN, BLOCK, TOKENS = 12288, 4096, 128, 2048
N_CORES = 8
OSH = OUT // N_CORES          # 1536 out rows per core
N_OT = OSH // 128             # 12 row-blocks of 128
N_B = IN // BLOCK             # 32 k-blocks of 128
N_NCH = TOKENS // 512         # 4 moving chunks of 512

_CACHED = {}


def _build_nc():
    import concourse.bacc as bacc
    import concourse.mybir as mybir
    import concourse.tile as tile
    from contextlib import ExitStack

    nc = bacc.Bacc("TRN2", target_bir_lowering=False)
    f16, f32, i32 = mybir.dt.float16, mybir.dt.float32, mybir.dt.int32

    pk_d = nc.dram_tensor("pk", [OSH, 2048], i32, kind="ExternalInput")
    xt_d = nc.dram_tensor("xt", [IN, TOKENS], f16, kind="ExternalInput")
    sv_d = nc.dram_tensor("sv", [128, N_OT * N_B], f32, kind="ExternalInput")
    zv_d = nc.dram_tensor("zv", [128, N_OT * N_B], f32, kind="ExternalInput")
    id_d = nc.dram_tensor("ident", [128, 128], f16, kind="ExternalInput")
    yt_d = nc.dram_tensor("yt", [OSH, TOKENS], f32, kind="ExternalOutput")

    RSH = mybir.AluOpType.logical_shift_right
    AND = mybir.AluOpType.bitwise_and
    ADD = mybir.AluOpType.add
    COPY = mybir.ActivationFunctionType.Copy

    with tile.TileContext(nc) as tc, ExitStack() as ctx:
        const = ctx.enter_context(tc.tile_pool(name="const", bufs=1))
        xpool = ctx.enter_context(tc.tile_pool(name="xpool", bufs=1))
        pkpool = ctx.enter_context(tc.tile_pool(name="pkpool", bufs=4))
        wtpool = ctx.enter_context(tc.tile_pool(name="wtpool", bufs=2))
        qpool = ctx.enter_context(tc.tile_pool(name="qpool", bufs=4))
        dpool = ctx.enter_context(tc.tile_pool(name="dpool", bufs=4))
        ypool = ctx.enter_context(tc.tile_pool(name="ypool", bufs=3))
        psw = ctx.enter_context(tc.tile_pool(name="psw", bufs=2, space="PSUM"))
        psy = ctx.enter_context(tc.tile_pool(name="psy", bufs=5, space="PSUM"))

        sv_sb = const.tile([128, N_OT * N_B], f32, name="sv_sb")
        zv_sb = const.tile([128, N_OT * N_B], f32, name="zv_sb")
        id_sb = const.tile([128, 128], f16, name="id_sb")
        nc.sync.dma_start(sv_sb[:], sv_d[:, :])
        nc.sync.dma_start(zv_sb[:], zv_d[:, :])
        nc.sync.dma_start(id_sb[:], id_d[:, :])

        # packed half-row-block tiles (16 k-blocks each), keyed (ot, half)
        pk_tiles = {}

        def load_packed(ot):
            for h in range(2):
                t = pkpool.tile([128, 1024], i32, name="pk_sb", tag="pk_sb")
                nc.sync.dma_start(
                    t[:], pk_d[ot * 128:(ot + 1) * 128, h * 1024:(h + 1) * 1024])
                pk_tiles[(ot, h)] = t

        load_packed(0)
        if N_OT > 1:
            load_packed(1)

        # resident x.T: one big SBUF tile, free index = b*2048 + t
        xt_sb = xpool.tile([128, N_B * 2048], f16, name="xt_sb")
        for b in range(N_B):
            nc.sync.dma_start(
                xt_sb[:, b * 2048:(b + 1) * 2048],
                xt_d[b * 128:(b + 1) * 128, :],
            )

        def make_wt(ot):
            """Produce the [128 i', 32*128 o'] fp16 stationary tiles for ot.

            Returns per-b emit closures so callers can interleave them with
            main matmuls.
            """
            wt = wtpool.tile([128, N_B * 128], f16, name="wt_sb", tag="wt_sb")
            # batched nibble extracts per half-row-block (bitwise ops cannot
            # cast, so stay int32): layout per half = [hi 1024 | lo 1024]
            qr3s = []
            for h in range(2):
                pk_sb = pk_tiles[(ot, h)]
                qraw = qpool.tile([128, 2048], i32, name="qraw", tag="qraw",
                                  bufs=2)
                nc.vector.tensor_scalar(qraw[:, 0:1024], pk_sb[:], 4, None, RSH)
                nc.vector.tensor_scalar(qraw[:, 1024:2048], pk_sb[:], 15, None, AND)
                qr3s.append(qraw[:].rearrange("p (h c) -> p h c", h=2))

            def emit_b(b):
                idx = ot * N_B + b
                qr3, lb = qr3s[b // 16], b % 16
                # convert + per-partition zos add in one arith op; input is
                # the two 64-wide chunks (hi, lo) of block b
                qn = qpool.tile([128, 128], f16, name="qn", tag="qn")
                qn3 = qn[:].rearrange("p (h c) -> p h c", h=2)
                nc.vector.tensor_scalar(
                    qn3[:, :, :], qr3[:, :, lb * 64:(lb + 1) * 64],
                    zv_sb[:, idx:idx + 1], None, ADD)
                ds = dpool.tile([128, 128], f16, name="ds", tag="ds")
                nc.scalar.activation(ds[:], id_sb[:], COPY,
                                     bias=0.0, scale=sv_sb[:, idx:idx + 1])
                pw = psw.tile([128, 128], f32, name="pw", tag="pw")
                nc.tensor.matmul(pw[:], lhsT=qn[:], rhs=ds[:],
                                 start=True, stop=True)
                nc.any.tensor_copy(wt[:, b * 128:(b + 1) * 128], pw[:])

            return wt, emit_b

        # prologue: dequantize ot=0 fully
        wt_cur, emit_cur = make_wt(0)
        for b in range(N_B):
            emit_cur(b)

        for ot in range(N_OT):
            if ot + 2 < N_OT:
                load_packed(ot + 2)
            if ot + 1 < N_OT:
                wt_next, emit_next = make_wt(ot + 1)
            else:
                wt_next, emit_next = None, None

            pys = [psy.tile([128, 512], f32, name="py", tag="py")
                   for _ in range(N_NCH)]
            for b in range(N_B):
                if emit_next is not None:
                    emit_next(b)
                for nch in range(N_NCH):
                    nc.tensor.matmul(
                        pys[nch][:],
                        lhsT=wt_cur[:, b * 128:(b + 1) * 128],
                        rhs=xt_sb[:, b * 2048 + nch * 512: b * 2048 + nch * 512 + 512],
                        start=(b == 0), stop=(b == N_B - 1))
            for nch in range(N_NCH):
                y_sb = ypool.tile([128, 512], f32, name="y_sb", tag="y_sb")
                nc.any.tensor_copy(y_sb[:], pys[nch][:])
                nc.sync.dma_start(
                    yt_d[ot * 128:(ot + 1) * 128, nch * 512:(nch + 1) * 512],
                    y_sb[:])
            wt_cur, emit_cur = wt_next, emit_next

    nc.compile()
    return nc


def _host_prep(x, packed, scales, zeros):
    # i' permutation within each 128-block: evens (hi nibbles) then odds
    perm = np.empty(BLOCK, dtype=np.int64)
    perm[:64] = np.arange(64) * 2
    perm[64:] = np.arange(64) * 2 + 1
    full_perm = (np.arange(IN) // BLOCK) * BLOCK
    full_perm = full_perm + np.tile(perm, IN // BLOCK)

    xt = np.ascontiguousarray(x.T)[full_perm].astype(np.float16)

    pk2 = packed.reshape(OUT, IN // 2).astype(np.int32)
    s2 = scales.reshape(OUT, N_B).astype(np.float32)
    zos2 = (zeros.astype(np.float64) / scales.astype(np.float64))
    zos2 = zos2.reshape(OUT, N_B).astype(np.float32)

    ident = np.eye(128, dtype=np.float16)

    in_maps = []
    for c in range(N_CORES):
        rows = slice(c * OSH, (c + 1) * OSH)
        sv = np.ascontiguousarray(
            s2[rows].reshape(N_OT, 128, N_B).transpose(1, 0, 2).reshape(128, N_OT * N_B))
        zv = np.ascontiguousarray(
            zos2[rows].reshape(N_OT, 128, N_B).transpose(1, 0, 2).reshape(128, N_OT * N_B))
        in_maps.append({
            "pk": np.ascontiguousarray(pk2[rows]),
            "xt": xt,
            "sv": sv,
            "zv": zv,
            "ident": ident,
        })
    return in_maps


def kernel(x, packed, scales, zeros, bias):
    from concourse.bass_utils import run_bass_kernel_spmd

    if "nc" not in _CACHED:
        _CACHED["nc"] = _build_nc()
    nc = _CACHED["nc"]

    in_maps = _host_prep(x, packed, scales, zeros)
    res = run_bass_kernel_spmd(nc, in_maps, core_ids=list(range(N_CORES)))
    yt = np.concatenate([res.results[c]["yt"] for c in range(N_CORES)], axis=0)
    y = yt.T.astype(np.float32) + bias.astype(np.float32)[None, :]
    return np.ascontiguousarray(y)
